# revision 2
# baseline (speedup 1.0000x reference)
"""PINN value+gradient+Hessian-diagonal kernel for Trainium2 (8 NeuronCores).

Math (per sample, scalar net u(x) with 4 tanh layers):
  forward:  z0 = x@W0, h_k = tanh(z_k), z_{k+1} = h_k@W_{k+1}, u = h3@Wout
            (all biases are zero by construction; bout added on host)
  tangent streams (dir i = unit vector e_i, D=3):
      z'_1,i = (W0_i-scaled W1)^T t0,  h'_k,i = t_k * z'_k,i,  t_k = 1-h_k^2
      z'_{k+1},i = W_{k+1}^T h'_k,i
  backward:  a3 = t3*Wout;  b_{k-1} = W_k^T a_k;  a_{k-1} = t_{k-1}*b_{k-1}
      grads = W0^T a0
  Hessian diagonal (exact identity):
      u''_i = -2 * sum_k (h_k . a_k) . (z'_k,i)^2
  Layer-0 term uses constant -2*(W0[i,:])^2 folded into a matmul stationary.

Device layout: activations [feat(part 128 x 2 chunks), batch(free N=256)],
fp16 in SBUF (weights fp16 stationaries; layer-0 stays f32r x f32r since
the PE forbids mixing 32-bit with 16-bit operands); PSUM f32 for matmul
outputs. One [7,N] PSUM accumulator per tile holds rows (hess0..2, u,
grad0..2); all accumulator matmuls write the full [7,N] region at base
partition 0 through zero-padded selector stationaries (col j of 7 selects
the output row), so a single start=True on the first dot-product matmul
replaces bank-clearing.

(1-h^2) factors are fused into consumers as (s-1)*x via
scalar_tensor_tensor (s = h^2): the resulting sign flips alternate per
layer, are absorbed by the q = zd^2 squares on the tangent path, and on
the backward path are folded into per-layer +/-2 dot-product selector
constants — t_k tensors (k>=1) are never materialized.

Engine assignment: Act = tanh x4 + q-squares x9 + output staging; DVE =
tangent/backward PSUM drains (hd, a), products (m, s), a3; Pool(GpSimd,
SBUF-only) = cc products + c0; PE = 86 matmuls/tile at 256-moving each.
Emission is software-pipelined: the tangent ladder is split per direction
(zd pool bufs=4, one bank each) so the three ladders hide each other's
DVE latency, and phase_b (backward + dot products) of tile t-2 is
interleaved chunk-wise into phase_a of tile t via generators.
"""

import numpy as np
from contextlib import ExitStack

import concourse.bass as bass
import concourse.bacc as bacc
import concourse.tile as tile
import concourse.mybir as mybir
from concourse.bass_utils import run_bass_kernel_spmd

F32 = mybir.dt.float32
F32R = mybir.dt.float32r
FP16 = mybir.dt.float16
AF = mybir.ActivationFunctionType
ALU = mybir.AluOpType

B, D, H = 65536, 3, 256
NCORES = 8
BLOC = B // NCORES          # 8192 samples per core
N = 256                     # batch tile width (free dim)
NT = BLOC // N              # tiles per core

_CACHE = {}


def _build(nt=NT, reps=1):
    NT_local = nt
    nc = bacc.Bacc("TRN2")

    bloc = NT_local * N
    xt = nc.dram_tensor("xt", [D, bloc], F32R, kind="ExternalInput")
    w0 = nc.dram_tensor("w0", [D, H], F32R, kind="ExternalInput")
    wf = nc.dram_tensor("wf", [128, 3, 2, 2, 128], FP16, kind="ExternalInput")
    wt1 = nc.dram_tensor("wt1", [128, 3, 2, 2, 128], FP16, kind="ExternalInput")
    wb = nc.dram_tensor("wb", [128, 3, 2, 2, 128], FP16, kind="ExternalInput")
    # [7,N]-accumulator stationaries: col i of 7 selects the output row.
    # m2sel[:, i, :]: col i = -2 (hess dir i); wosel[:, c, :]: col 3 = Wout
    # chunk c (u); q0sel[:, c, :]: cols 0..2 = -2*(W0^2)^T (layer-0 hess);
    # w0sel[:, c, :]: cols 4..6 = W0^T (grads).
    m2sel = nc.dram_tensor("m2sel", [128, 2, 3, 7], FP16, kind="ExternalInput")
    wosel = nc.dram_tensor("wosel", [128, 2, 7], FP16, kind="ExternalInput")
    q0sel = nc.dram_tensor("q0sel", [128, 2, 7], FP16, kind="ExternalInput")
    w0sel = nc.dram_tensor("w0sel", [128, 2, 7], FP16, kind="ExternalInput")
    won = nc.dram_tensor("won", [128, 2], F32, kind="ExternalInput")      # -Wout (f32)
    wop = nc.dram_tensor("wop", [128, 2], F32, kind="ExternalInput")      # +Wout (f32)
    out = nc.dram_tensor("out", [7, bloc], F32, kind="ExternalOutput")

    with tile.TileContext(nc) as tc, ExitStack() as ctx:
        const = ctx.enter_context(tc.tile_pool(name="const", bufs=1))
        act = ctx.enter_context(tc.tile_pool(name="act", bufs=2))
        pz = ctx.enter_context(tc.tile_pool(name="pz", bufs=2, space="PSUM"))
        ps_zd = ctx.enter_context(tc.tile_pool(name="ps_zd", bufs=4, space="PSUM"))
        ps_acc = ctx.enter_context(tc.tile_pool(name="ps_acc", bufs=2, space="PSUM"))

        xall = const.tile([D, NT_local * N], F32R)
        nc.sync.dma_start(xall[:], xt[:])
        w0s = const.tile([D, H], F32R)
        nc.sync.dma_start(w0s[:], w0[:])
        wfs = const.tile([128, 3, 2, 2, 128], FP16)
        nc.sync.dma_start(wfs[:], wf[:])
        wt1s = const.tile([128, 3, 2, 2, 128], FP16)
        nc.sync.dma_start(wt1s[:], wt1[:])
        wbs = const.tile([128, 3, 2, 2, 128], FP16)
        nc.sync.dma_start(wbs[:], wb[:])
        m2sels = const.tile([128, 2, 3, 7], FP16)
        nc.sync.dma_start(m2sels[:], m2sel[:])
        wosels = const.tile([128, 2, 7], FP16)
        nc.sync.dma_start(wosels[:], wosel[:])
        q0sels = const.tile([128, 2, 7], FP16)
        nc.sync.dma_start(q0sels[:], q0sel[:])
        w0sels = const.tile([128, 2, 7], FP16)
        nc.sync.dma_start(w0sels[:], w0sel[:])
        wons = const.tile([128, 2], F32)
        nc.sync.dma_start(wons[:], won[:])
        wops = const.tile([128, 2], F32)
        nc.sync.dma_start(wops[:], wop[:])

        def tt(eng, out_ap, in0, in1, op=ALU.mult):
            eng.tensor_tensor(out=out_ap, in0=in0, in1=in1, op=op)

        def sm1(eng, out_ap, s_in, in1):
            # out = (s - 1) * in1   (= -(1-s)*in1; sign tracked by caller)
            eng.scalar_tensor_tensor(out=out_ap, in0=s_in, scalar=1.0, in1=in1,
                                     op0=ALU.subtract, op1=ALU.mult)

        state = {}

        def phase_a(it):
            """forward + tangents of tile it. Generator: yields after each
            tangent-layer chunk so phase_b(it-1) work interleaves into the
            per-engine instruction streams; final state lands in state[it]."""
            col = slice(it * N, (it + 1) * N)
            xtile = xall[:, col]

            z = pz.tile([128, 2, N], F32, name="z0", tag="pz")
            for mc in range(2):
                nc.tensor.matmul(z[:, mc, :], w0s[:, mc * 128:(mc + 1) * 128],
                                 xtile, start=True, stop=True)
            h = [None] * 4
            t = [None] * 3
            q = [None] * 4
            h[0] = act.tile([128, 2, N], FP16, name="h0", tag="h0", bufs=6)
            nc.scalar.activation(h[0][:], z[:], AF.Tanh)
            s = [None] * 4
            s[0] = act.tile([128, 2, N], FP16, name="s0", tag="s0", bufs=6)
            tt(nc.vector, s[0][:], h[0][:], h[0][:])
            t[0] = act.tile([128, 2, N], FP16, name="t0", tag="t0", bufs=6)
            nc.vector.tensor_scalar(out=t[0][:], in0=s[0][:], scalar1=-1.0,
                                    scalar2=1.0, op0=ALU.mult, op1=ALU.add)

            hd = None
            a3 = None
            for k in (1, 2, 3):
                l = k - 1
                z = pz.tile([128, 2, N], F32, name=f"z{k}", tag="pz")
                for mc in range(2):
                    for kc in range(2):
                        nc.tensor.matmul(z[:, mc, :], wfs[:, l, kc, mc, :],
                                         h[k - 1][:, kc, :],
                                         start=(kc == 0), stop=(kc == 1))
                h[k] = act.tile([128, 2, N], FP16, name=f"h{k}", tag=f"h{k}", bufs=6)
                nc.scalar.activation(h[k][:], z[:], AF.Tanh)
                if k < 3:
                    s[k] = act.tile([128, 2, N], FP16, name=f"s{k}", tag=f"s{k}", bufs=6)
                    if k == 1:
                        nc.scalar.activation(s[k][:], h[k][:], AF.Square)
                    else:
                        tt(nc.vector, s[k][:], h[k][:], h[k][:])
                else:
                    s[3] = act.tile([128, 2, N], FP16, name="s3", tag="s3", bufs=6)
                    tt(nc.gpsimd, s[3][:], h[3][:], h[3][:])

                zds = []
                new_hd = [None] * 3
                q[k] = act.tile([128, 3, 2, N], FP16, name=f"q{k}", tag=f"q{k}", bufs=4)
                for i in range(3):
                    zd = ps_zd.tile([128, 2, N], F32, name=f"zd{k}_{i}", tag="zd")
                    zds.append(zd)
                    for mc in range(2):
                        for kc in range(2):
                            if k == 1:
                                nc.tensor.matmul(zd[:, mc, :],
                                                 wt1s[:, i, kc, mc, :],
                                                 t[0][:, kc, :],
                                                 start=(kc == 0), stop=(kc == 1))
                            else:
                                nc.tensor.matmul(zd[:, mc, :],
                                                 wfs[:, l, kc, mc, :],
                                                 hd[i][:, kc, :],
                                                 start=(kc == 0), stop=(kc == 1))
                for i in range(3):
                    nc.scalar.activation(q[k][:, i, :, :], zds[i][:], AF.Square)
                    if k < 3:
                        # hd' = (s_k - 1) * zd  (sign alternates; q squares absorb it)
                        new_hd[i] = act.tile([128, 2, N], FP16,
                                             name=f"hd{k}_{i}", tag=f"hd{i}", bufs=4)
                        sm1(nc.vector, new_hd[i][:], s[k][:], zds[i][:])
                hd = new_hd
                if k == 3:
                    state[it] = (col, h, s, q)
                yield

        def phase_b(it):
            """backward + dot products of tile it. Generator: 4 chunks.
            Sign ledger: a3'=-a3, a2'=+a2, a1'=-a1, a0'=+a0 (from the (s-1)
            fusion); cc_k inherits a_k's sign; m2sel[:,0] = +2 (k=3,1),
            m2sel[:,1] = -2 (k=2)."""
            col, h, s, q = state.pop(it)
            # a3' = -t3*Wout = s3*w - w  (negated; absorbed in m2sel signs)
            a3 = act.tile([128, 2, N], FP16, name="a3", tag="a3", bufs=4)
            for c in range(2):
                nc.vector.tensor_scalar(out=a3[:, c, :], in0=s[3][:, c, :],
                                        scalar1=wops[:, c:c + 1],
                                        scalar2=wons[:, c:c + 1],
                                        op0=ALU.mult, op1=ALU.add)
            a = a3
            acc = ps_acc.tile([7, N], F32, name="acc", tag="acc")
            first = True
            for k in (3, 2, 1):
                cc = act.tile([128, 2, N], FP16, name=f"cc{k}", tag="cc", bufs=4)
                tt(nc.gpsimd, cc[:], h[k][:], a[:])
                ccb = cc[:].unsqueeze(1).broadcast_to([128, 3, 2, N])
                m = act.tile([128, 3, 2, N], FP16, name=f"m{k}", tag=f"m{k}", bufs=4)
                tt(nc.vector, m[:], q[k][:], ccb)
                bk = pz.tile([128, 2, N], F32, name=f"bk{k}", tag="pz")
                for mc in range(2):
                    for kc in range(2):
                        nc.tensor.matmul(bk[:, mc, :], wbs[:, k - 1, kc, mc, :],
                                         a[:, kc, :],
                                         start=(kc == 0), stop=(kc == 1))
                a = act.tile([128, 2, N], FP16, name=f"a{k - 1}", tag="a", bufs=4)
                sm1(nc.vector, a[:], s[k - 1][:], bk[:])
                # drain this k's dot products into acc right away
                sgn = 0 if k != 2 else 1
                for i in range(3):
                    for c in range(2):
                        nc.tensor.matmul(acc[:], m2sels[:, sgn, i, :],
                                         m[:, i, c, :],
                                         start=first, stop=False,
                                         skip_group_check=True)
                        first = False
                if k == 3:
                    for c in range(2):
                        nc.tensor.matmul(acc[:], wosels[:, c, :], h[3][:, c, :],
                                         start=False, stop=False,
                                         skip_group_check=True)
                yield
            c0 = act.tile([128, 2, N], FP16, name="c0", tag="c0", bufs=4)
            tt(nc.gpsimd, c0[:], h[0][:], a[:])

            for c in range(2):
                nc.tensor.matmul(acc[:], q0sels[:, c, :], c0[:, c, :],
                                 start=False, stop=False,
                                 skip_group_check=True)
            for c in range(2):
                nc.tensor.matmul(acc[:], w0sels[:, c, :], a[:, c, :],
                                 start=False, stop=(c == 1),
                                 skip_group_check=True)

            stg = act.tile([7, N], F32, name="stg", tag="stg", bufs=4)
            nc.scalar.copy(stg[:], acc[:])
            nc.sync.dma_start(out[:, col], stg[:])

        def drive(gen):
            if gen is None:
                return None
            try:
                next(gen)
                return gen
            except StopIteration:
                return None

        for rep in range(reps):
            queue = {}
            # fill prologue: interleave A(0) and A(1) chunk-by-chunk so each
            # fills the other's dependency-chain gaps (no B partner exists yet)
            if NT_local >= 2:
                ga0, ga1 = phase_a(0), phase_a(1)
                drive(ga0)
                drive(ga1)
                drive(ga0)
                drive(ga1)
                drive(ga0)
                gb0 = phase_b(0)
                drive(gb0)          # B1(0) between A3(0) and A3(1)
                drive(ga1)
                queue[0] = gb0
                queue[1] = phase_b(1)
                start_it = 2
            else:
                start_it = 0
            for it in range(start_it, NT_local):
                ga = phase_a(it)
                gb = queue.pop(it - 2, None)
                for _ in range(3):          # 3 layer chunks of A
                    drive(ga)
                    gb = drive(gb)
                gb = drive(gb)              # B final chunk (dd+stg)
                assert gb is None
                queue[it] = phase_b(it)
            gens = [queue[it] for it in sorted(queue)]
            while gens:
                gens = [g for g in (drive(g) for g in gens) if g is not None]

    nc.compile()
    return nc


def _host_pack(inputs):
    x = np.ascontiguousarray(np.asarray(inputs["x"], np.float32))
    W = [np.asarray(inputs[f"W{i}"], np.float32) for i in range(4)]
    Wout = np.asarray(inputs["Wout"], np.float32)
    bout = np.asarray(inputs["bout"], np.float32)

    def pack_w(w):   # [256,256] -> [128, 2(kc), 2(mc), 128]
        return np.ascontiguousarray(w.reshape(2, 128, 2, 128).transpose(1, 0, 2, 3))

    wf = np.ascontiguousarray(np.stack([pack_w(W[1]), pack_w(W[2]), pack_w(W[3])], axis=1))
    wbk = np.ascontiguousarray(np.stack(
        [pack_w(W[1].T.copy()), pack_w(W[2].T.copy()), pack_w(W[3].T.copy())], axis=1))

    def pack_kd(wkd):   # [256, D] -> [128, 2, D]
        return np.ascontiguousarray(wkd.reshape(2, 128, D).transpose(1, 0, 2))

    w0t = pack_kd(W[0].T.copy())                      # [128, 2, 3] grads lhsT
    q0t = pack_kd((-2.0 * W[0].astype(np.float64) ** 2).astype(np.float32).T.copy())
    wt1 = np.ascontiguousarray(np.stack(
        [pack_w((W[0][i, :][:, None] * W[1]).astype(np.float32)) for i in range(3)],
        axis=1))
    wo = np.ascontiguousarray(Wout[:, 0].reshape(2, 128).T)     # [128, 2]
    xtp = np.ascontiguousarray(x.T)                             # [D, B]

    # [7,N]-accumulator selector stationaries (see _build for row layout).
    # m2sel[:, 0]: +2 (layers 3,1 whose cc carries a negated a');
    # m2sel[:, 1]: -2 (layer 2, true-sign cc).
    m2sel = np.zeros((128, 2, 3, 7), np.float16)
    for i in range(3):
        m2sel[:, 0, i, i] = 2.0
        m2sel[:, 1, i, i] = -2.0
    wosel = np.zeros((128, 2, 7), np.float32)
    wosel[:, :, 3] = wo
    q0sel = np.zeros((128, 2, 7), np.float32)
    q0sel[:, :, 0:3] = q0t
    w0sel = np.zeros((128, 2, 7), np.float32)
    w0sel[:, :, 4:7] = w0t

    shared = dict(w0=W[0], wf=wf.astype(np.float16), wb=wbk.astype(np.float16),
                  wt1=wt1.astype(np.float16), m2sel=m2sel,
                  wosel=wosel.astype(np.float16), q0sel=q0sel.astype(np.float16),
                  w0sel=w0sel.astype(np.float16),
                  won=np.ascontiguousarray(-wo), wop=wo.copy())
    return xtp, shared, float(bout[0])


LAST_EXEC_NS = None


def kernel(**inputs):
    global LAST_EXEC_NS
    import os
    if "nc" not in _CACHE:
        _CACHE["nc"] = _build()
    nc = _CACHE["nc"]

    xt, shared, bout = _host_pack(inputs)
    in_maps = []
    for c in range(NCORES):
        m = dict(shared)
        m["xt"] = np.ascontiguousarray(xt[:, c * BLOC:(c + 1) * BLOC])
        in_maps.append(m)

    trace = bool(int(os.environ.get("BASS_PINN_TRACE", "0")))
    res = run_bass_kernel_spmd(nc, in_maps, core_ids=list(range(NCORES)),
                               trace=trace)
    if res.exec_time_ns is not None:
        LAST_EXEC_NS = res.exec_time_ns
    if res.instructions_and_trace is not None:
        print("trace:", res.instructions_and_trace[1])
    full = np.concatenate([res.results[c]["out"] for c in range(NCORES)], axis=1)
    y = np.empty((full.shape[1], 7), np.float32)
    y[:, 0] = full[3] + np.float32(bout)
    y[:, 1:4] = full[4:7].T
    y[:, 4:7] = full[0:3].T
    return y



# revision 4
# speedup vs baseline: 1.0059x; 1.0059x over previous
"""PINN value+gradient+Hessian-diagonal kernel for Trainium2 (8 NeuronCores).

Math (per sample, scalar net u(x) with 4 tanh layers):
  forward:  z0 = x@W0, h_k = tanh(z_k), z_{k+1} = h_k@W_{k+1}, u = h3@Wout
            (all biases are zero by construction; bout added on host)
  tangent streams (dir i = unit vector e_i, D=3):
      z'_1,i = (W0_i-scaled W1)^T t0,  h'_k,i = t_k * z'_k,i,  t_k = 1-h_k^2
      z'_{k+1},i = W_{k+1}^T h'_k,i
  backward:  a3 = t3*Wout;  b_{k-1} = W_k^T a_k;  a_{k-1} = t_{k-1}*b_{k-1}
      grads = W0^T a0
  Hessian diagonal (exact identity):
      u''_i = -2 * sum_k (h_k . a_k) . (z'_k,i)^2
  Layer-0 term uses constant -2*(W0[i,:])^2 folded into a matmul stationary.

Device layout: activations [feat(part 128 x 2 chunks), batch(free N=256)],
fp16 in SBUF (weights fp16 stationaries; layer-0 stays f32r x f32r since
the PE forbids mixing 32-bit with 16-bit operands); PSUM f32 for matmul
outputs. One [7,N] PSUM accumulator per tile holds rows (hess0..2, u,
grad0..2); all accumulator matmuls write the full [7,N] region at base
partition 0 through zero-padded selector stationaries (col j of 7 selects
the output row), so a single start=True on the first dot-product matmul
replaces bank-clearing.

(1-h^2) factors are fused into consumers as (s-1)*x via
scalar_tensor_tensor (s = h^2): the resulting sign flips alternate per
layer, are absorbed by the q = zd^2 squares on the tangent path, and on
the backward path are folded into per-layer +/-2 dot-product selector
constants — t_k tensors (k>=1) are never materialized.

Engine assignment: Act = tanh x4 + q-squares x9 + output staging; DVE =
tangent/backward PSUM drains (hd, a), products (m, s), a3; Pool(GpSimd,
SBUF-only) = cc products + c0; PE = 86 matmuls/tile at 256-moving each.
Emission is software-pipelined: the tangent ladder is split per direction
(zd pool bufs=4, one bank each) so the three ladders hide each other's
DVE latency, and phase_b (backward + dot products) of tile t-2 is
interleaved chunk-wise into phase_a of tile t via generators.

Boundary overlap (-4% vs the plain loop): all of x is prefetched in one
DMA issued before the weight tables (the startup DMA queue otherwise
stalls tile 0 by ~5us); 4 dummy matmuls during that DMA wait start the
PE P-state ramp clock early; phase_a(0)/phase_a(1) interleave in a fill
prologue with B1(0) driven between their last chunks; the two drain-tail
phase_b generators alternate chunk-by-chunk instead of running serially.
"""

import numpy as np
from contextlib import ExitStack

import concourse.bass as bass
import concourse.bacc as bacc
import concourse.tile as tile
import concourse.mybir as mybir
from concourse.bass_utils import run_bass_kernel_spmd

F32 = mybir.dt.float32
F32R = mybir.dt.float32r
FP16 = mybir.dt.float16
AF = mybir.ActivationFunctionType
ALU = mybir.AluOpType

B, D, H = 65536, 3, 256
NCORES = 8
BLOC = B // NCORES          # 8192 samples per core
N = 256                     # batch tile width (free dim)
NT = BLOC // N              # tiles per core

_CACHE = {}


def _build(nt=NT, reps=1):
    NT_local = nt
    nc = bacc.Bacc("TRN2")

    bloc = NT_local * N
    xt = nc.dram_tensor("xt", [D, bloc], F32R, kind="ExternalInput")
    w0 = nc.dram_tensor("w0", [D, H], F32R, kind="ExternalInput")
    wf = nc.dram_tensor("wf", [128, 3, 2, 2, 128], FP16, kind="ExternalInput")
    wt1 = nc.dram_tensor("wt1", [128, 3, 2, 2, 128], FP16, kind="ExternalInput")
    wb = nc.dram_tensor("wb", [128, 3, 2, 2, 128], FP16, kind="ExternalInput")
    # [7,N]-accumulator stationaries: col i of 7 selects the output row.
    # m2sel[:, i, :]: col i = -2 (hess dir i); wosel[:, c, :]: col 3 = Wout
    # chunk c (u); q0sel[:, c, :]: cols 0..2 = -2*(W0^2)^T (layer-0 hess);
    # w0sel[:, c, :]: cols 4..6 = W0^T (grads).
    m2sel = nc.dram_tensor("m2sel", [128, 2, 3, 7], FP16, kind="ExternalInput")
    wosel = nc.dram_tensor("wosel", [128, 2, 7], FP16, kind="ExternalInput")
    q0sel = nc.dram_tensor("q0sel", [128, 2, 7], FP16, kind="ExternalInput")
    w0sel = nc.dram_tensor("w0sel", [128, 2, 7], FP16, kind="ExternalInput")
    won = nc.dram_tensor("won", [128, 2], F32, kind="ExternalInput")      # -Wout (f32)
    wop = nc.dram_tensor("wop", [128, 2], F32, kind="ExternalInput")      # +Wout (f32)
    out = nc.dram_tensor("out", [7, bloc], F32, kind="ExternalOutput")

    with tile.TileContext(nc) as tc, ExitStack() as ctx:
        const = ctx.enter_context(tc.tile_pool(name="const", bufs=1))
        act = ctx.enter_context(tc.tile_pool(name="act", bufs=2))
        pz = ctx.enter_context(tc.tile_pool(name="pz", bufs=2, space="PSUM"))
        ps_zd = ctx.enter_context(tc.tile_pool(name="ps_zd", bufs=4, space="PSUM"))
        ps_acc = ctx.enter_context(tc.tile_pool(name="ps_acc", bufs=2, space="PSUM"))

        xall = const.tile([D, NT_local * N], F32R)
        nc.sync.dma_start(xall[:], xt[:])
        w0s = const.tile([D, H], F32R)
        nc.sync.dma_start(w0s[:], w0[:])
        wfs = const.tile([128, 3, 2, 2, 128], FP16)
        nc.sync.dma_start(wfs[:], wf[:])
        wt1s = const.tile([128, 3, 2, 2, 128], FP16)
        nc.sync.dma_start(wt1s[:], wt1[:])
        wbs = const.tile([128, 3, 2, 2, 128], FP16)
        nc.sync.dma_start(wbs[:], wb[:])
        m2sels = const.tile([128, 2, 3, 7], FP16)
        nc.sync.dma_start(m2sels[:], m2sel[:])
        wosels = const.tile([128, 2, 7], FP16)
        nc.sync.dma_start(wosels[:], wosel[:])
        q0sels = const.tile([128, 2, 7], FP16)
        nc.sync.dma_start(q0sels[:], q0sel[:])
        w0sels = const.tile([128, 2, 7], FP16)
        nc.sync.dma_start(w0sels[:], w0sel[:])
        wons = const.tile([128, 2], F32)
        nc.sync.dma_start(wons[:], won[:])
        wops = const.tile([128, 2], F32)
        nc.sync.dma_start(wops[:], wop[:])

        def tt(eng, out_ap, in0, in1, op=ALU.mult):
            eng.tensor_tensor(out=out_ap, in0=in0, in1=in1, op=op)

        def sm1(eng, out_ap, s_in, in1):
            # out = (s - 1) * in1   (= -(1-s)*in1; sign tracked by caller)
            eng.scalar_tensor_tensor(out=out_ap, in0=s_in, scalar=1.0, in1=in1,
                                     op0=ALU.subtract, op1=ALU.mult)

        # PE warm-up: dummy matmuls during the initial DMA wait start the
        # P-state ramp clock early (PE would otherwise idle until x lands)
        warm = const.tile([128, 256], FP16)
        nc.vector.memset(warm[:], 0.0)
        warm_ps = pz.tile([128, 2, 256], F32, name="warm_ps", tag="pz")
        for _ in range(4):
            nc.tensor.matmul(warm_ps[:, 0, :], warm[:, 0:128], warm[:],
                             start=True, stop=True)

        state = {}

        def phase_a(it):
            """forward + tangents of tile it. Generator: yields after each
            tangent-layer chunk so phase_b(it-1) work interleaves into the
            per-engine instruction streams; final state lands in state[it]."""
            col = slice(it * N, (it + 1) * N)
            xtile = xall[:, col]

            z = pz.tile([128, 2, N], F32, name="z0", tag="pz")
            for mc in range(2):
                nc.tensor.matmul(z[:, mc, :], w0s[:, mc * 128:(mc + 1) * 128],
                                 xtile, start=True, stop=True)
            h = [None] * 4
            t = [None] * 3
            q = [None] * 4
            h[0] = act.tile([128, 2, N], FP16, name="h0", tag="h0", bufs=6)
            nc.scalar.activation(h[0][:], z[:], AF.Tanh)
            s = [None] * 4
            s[0] = act.tile([128, 2, N], FP16, name="s0", tag="s0", bufs=6)
            tt(nc.vector, s[0][:], h[0][:], h[0][:])
            t[0] = act.tile([128, 2, N], FP16, name="t0", tag="t0", bufs=6)
            nc.vector.tensor_scalar(out=t[0][:], in0=s[0][:], scalar1=-1.0,
                                    scalar2=1.0, op0=ALU.mult, op1=ALU.add)

            hd = None
            a3 = None
            for k in (1, 2, 3):
                l = k - 1
                z = pz.tile([128, 2, N], F32, name=f"z{k}", tag="pz")
                for mc in range(2):
                    for kc in range(2):
                        nc.tensor.matmul(z[:, mc, :], wfs[:, l, kc, mc, :],
                                         h[k - 1][:, kc, :],
                                         start=(kc == 0), stop=(kc == 1))
                h[k] = act.tile([128, 2, N], FP16, name=f"h{k}", tag=f"h{k}", bufs=6)
                nc.scalar.activation(h[k][:], z[:], AF.Tanh)
                if k < 3:
                    s[k] = act.tile([128, 2, N], FP16, name=f"s{k}", tag=f"s{k}", bufs=6)
                    if k == 1:
                        nc.scalar.activation(s[k][:], h[k][:], AF.Square)
                    else:
                        tt(nc.vector, s[k][:], h[k][:], h[k][:])
                else:
                    s[3] = act.tile([128, 2, N], FP16, name="s3", tag="s3", bufs=6)
                    tt(nc.gpsimd, s[3][:], h[3][:], h[3][:])

                zds = []
                new_hd = [None] * 3
                q[k] = act.tile([128, 3, 2, N], FP16, name=f"q{k}", tag=f"q{k}", bufs=4)
                for i in range(3):
                    zd = ps_zd.tile([128, 2, N], F32, name=f"zd{k}_{i}", tag="zd")
                    zds.append(zd)
                    for mc in range(2):
                        for kc in range(2):
                            if k == 1:
                                nc.tensor.matmul(zd[:, mc, :],
                                                 wt1s[:, i, kc, mc, :],
                                                 t[0][:, kc, :],
                                                 start=(kc == 0), stop=(kc == 1))
                            else:
                                nc.tensor.matmul(zd[:, mc, :],
                                                 wfs[:, l, kc, mc, :],
                                                 hd[i][:, kc, :],
                                                 start=(kc == 0), stop=(kc == 1))
                for i in range(3):
                    nc.scalar.activation(q[k][:, i, :, :], zds[i][:], AF.Square)
                    if k < 3:
                        # hd' = (s_k - 1) * zd  (sign alternates; q squares absorb it)
                        new_hd[i] = act.tile([128, 2, N], FP16,
                                             name=f"hd{k}_{i}", tag=f"hd{i}", bufs=4)
                        sm1(nc.vector, new_hd[i][:], s[k][:], zds[i][:])
                hd = new_hd
                if k == 3:
                    state[it] = (col, h, s, q)
                yield

        def phase_b(it):
            """backward + dot products of tile it. Generator: 4 chunks.
            Sign ledger: a3'=-a3, a2'=+a2, a1'=-a1, a0'=+a0 (from the (s-1)
            fusion); cc_k inherits a_k's sign; m2sel[:,0] = +2 (k=3,1),
            m2sel[:,1] = -2 (k=2)."""
            col, h, s, q = state.pop(it)
            # a3' = -t3*Wout = s3*w - w  (negated; absorbed in m2sel signs)
            a3 = act.tile([128, 2, N], FP16, name="a3", tag="a3", bufs=6)
            for c in range(2):
                nc.vector.tensor_scalar(out=a3[:, c, :], in0=s[3][:, c, :],
                                        scalar1=wops[:, c:c + 1],
                                        scalar2=wons[:, c:c + 1],
                                        op0=ALU.mult, op1=ALU.add)
            a = a3
            acc = ps_acc.tile([7, N], F32, name="acc", tag="acc")
            first = True
            for k in (3, 2, 1):
                cc = act.tile([128, 2, N], FP16, name=f"cc{k}", tag="cc", bufs=6)
                tt(nc.gpsimd, cc[:], h[k][:], a[:])
                ccb = cc[:].unsqueeze(1).broadcast_to([128, 3, 2, N])
                m = act.tile([128, 3, 2, N], FP16, name=f"m{k}", tag=f"m{k}", bufs=4)
                tt(nc.vector, m[:], q[k][:], ccb)
                bk = pz.tile([128, 2, N], F32, name=f"bk{k}", tag="pz")
                for mc in range(2):
                    for kc in range(2):
                        nc.tensor.matmul(bk[:, mc, :], wbs[:, k - 1, kc, mc, :],
                                         a[:, kc, :],
                                         start=(kc == 0), stop=(kc == 1))
                a = act.tile([128, 2, N], FP16, name=f"a{k - 1}", tag="a", bufs=6)
                sm1(nc.vector, a[:], s[k - 1][:], bk[:])
                # drain this k's dot products into acc right away
                sgn = 0 if k != 2 else 1
                for i in range(3):
                    for c in range(2):
                        nc.tensor.matmul(acc[:], m2sels[:, sgn, i, :],
                                         m[:, i, c, :],
                                         start=first, stop=False,
                                         skip_group_check=True)
                        first = False
                if k == 3:
                    for c in range(2):
                        nc.tensor.matmul(acc[:], wosels[:, c, :], h[3][:, c, :],
                                         start=False, stop=False,
                                         skip_group_check=True)
                yield
            c0 = act.tile([128, 2, N], FP16, name="c0", tag="c0", bufs=4)
            tt(nc.gpsimd, c0[:], h[0][:], a[:])

            for c in range(2):
                nc.tensor.matmul(acc[:], q0sels[:, c, :], c0[:, c, :],
                                 start=False, stop=False,
                                 skip_group_check=True)
            for c in range(2):
                nc.tensor.matmul(acc[:], w0sels[:, c, :], a[:, c, :],
                                 start=False, stop=(c == 1),
                                 skip_group_check=True)

            stg = act.tile([7, N], F32, name="stg", tag="stg", bufs=4)
            nc.scalar.copy(stg[:], acc[:])
            nc.sync.dma_start(out[:, col], stg[:])

        def drive(gen):
            if gen is None:
                return None
            try:
                next(gen)
                return gen
            except StopIteration:
                return None

        for rep in range(reps):
            queue = {}
            # fill prologue: interleave A(0) and A(1) chunk-by-chunk so each
            # fills the other's dependency-chain gaps (no B partner exists yet)
            if NT_local >= 2:
                ga0, ga1 = phase_a(0), phase_a(1)
                drive(ga0)
                drive(ga1)
                drive(ga0)
                drive(ga1)
                drive(ga0)
                gb0 = phase_b(0)
                drive(gb0)          # B1(0) between A3(0) and A3(1)
                drive(ga1)
                queue[0] = gb0
                queue[1] = phase_b(1)
                start_it = 2
            else:
                start_it = 0
            for it in range(start_it, NT_local):
                ga = phase_a(it)
                gb = queue.pop(it - 2, None)
                for _ in range(3):          # 3 layer chunks of A
                    drive(ga)
                    gb = drive(gb)
                gb = drive(gb)              # B final chunk (dd+stg)
                assert gb is None
                queue[it] = phase_b(it)
            gens = [queue[it] for it in sorted(queue)]
            while gens:
                gens = [g for g in (drive(g) for g in gens) if g is not None]

    nc.compile()
    return nc


def _host_pack(inputs):
    x = np.ascontiguousarray(np.asarray(inputs["x"], np.float32))
    W = [np.asarray(inputs[f"W{i}"], np.float32) for i in range(4)]
    Wout = np.asarray(inputs["Wout"], np.float32)
    bout = np.asarray(inputs["bout"], np.float32)

    def pack_w(w):   # [256,256] -> [128, 2(kc), 2(mc), 128]
        return np.ascontiguousarray(w.reshape(2, 128, 2, 128).transpose(1, 0, 2, 3))

    wf = np.ascontiguousarray(np.stack([pack_w(W[1]), pack_w(W[2]), pack_w(W[3])], axis=1))
    wbk = np.ascontiguousarray(np.stack(
        [pack_w(W[1].T.copy()), pack_w(W[2].T.copy()), pack_w(W[3].T.copy())], axis=1))

    def pack_kd(wkd):   # [256, D] -> [128, 2, D]
        return np.ascontiguousarray(wkd.reshape(2, 128, D).transpose(1, 0, 2))

    w0t = pack_kd(W[0].T.copy())                      # [128, 2, 3] grads lhsT
    q0t = pack_kd((-2.0 * W[0].astype(np.float64) ** 2).astype(np.float32).T.copy())
    wt1 = np.ascontiguousarray(np.stack(
        [pack_w((W[0][i, :][:, None] * W[1]).astype(np.float32)) for i in range(3)],
        axis=1))
    wo = np.ascontiguousarray(Wout[:, 0].reshape(2, 128).T)     # [128, 2]
    xtp = np.ascontiguousarray(x.T)                             # [D, B]

    # [7,N]-accumulator selector stationaries (see _build for row layout).
    # m2sel[:, 0]: +2 (layers 3,1 whose cc carries a negated a');
    # m2sel[:, 1]: -2 (layer 2, true-sign cc).
    m2sel = np.zeros((128, 2, 3, 7), np.float16)
    for i in range(3):
        m2sel[:, 0, i, i] = 2.0
        m2sel[:, 1, i, i] = -2.0
    wosel = np.zeros((128, 2, 7), np.float32)
    wosel[:, :, 3] = wo
    q0sel = np.zeros((128, 2, 7), np.float32)
    q0sel[:, :, 0:3] = q0t
    w0sel = np.zeros((128, 2, 7), np.float32)
    w0sel[:, :, 4:7] = w0t

    shared = dict(w0=W[0], wf=wf.astype(np.float16), wb=wbk.astype(np.float16),
                  wt1=wt1.astype(np.float16), m2sel=m2sel,
                  wosel=wosel.astype(np.float16), q0sel=q0sel.astype(np.float16),
                  w0sel=w0sel.astype(np.float16),
                  won=np.ascontiguousarray(-wo), wop=wo.copy())
    return xtp, shared, float(bout[0])


LAST_EXEC_NS = None


def kernel(**inputs):
    global LAST_EXEC_NS
    import os
    if "nc" not in _CACHE:
        _CACHE["nc"] = _build()
    nc = _CACHE["nc"]

    xt, shared, bout = _host_pack(inputs)
    in_maps = []
    for c in range(NCORES):
        m = dict(shared)
        m["xt"] = np.ascontiguousarray(xt[:, c * BLOC:(c + 1) * BLOC])
        in_maps.append(m)

    trace = bool(int(os.environ.get("BASS_PINN_TRACE", "0")))
    res = run_bass_kernel_spmd(nc, in_maps, core_ids=list(range(NCORES)),
                               trace=trace)
    if res.exec_time_ns is not None:
        LAST_EXEC_NS = res.exec_time_ns
    if res.instructions_and_trace is not None:
        print("trace:", res.instructions_and_trace[1])
    full = np.concatenate([res.results[c]["out"] for c in range(NCORES)], axis=1)
    y = np.empty((full.shape[1], 7), np.float32)
    y[:, 0] = full[3] + np.float32(bout)
    y[:, 1:4] = full[4:7].T
    y[:, 4:7] = full[0:3].T
    return y



# revision 5
# speedup vs baseline: 1.0119x; 1.0060x over previous
"""PINN value+gradient+Hessian-diagonal kernel for Trainium2 (8 NeuronCores).

Math (per sample, scalar net u(x) with 4 tanh layers):
  forward:  z0 = x@W0, h_k = tanh(z_k), z_{k+1} = h_k@W_{k+1}, u = h3@Wout
            (all biases are zero by construction; bout added on host)
  tangent streams (dir i = unit vector e_i, D=3):
      z'_1,i = (W0_i-scaled W1)^T t0,  h'_k,i = t_k * z'_k,i,  t_k = 1-h_k^2
      z'_{k+1},i = W_{k+1}^T h'_k,i
  backward:  a3 = t3*Wout;  b_{k-1} = W_k^T a_k;  a_{k-1} = t_{k-1}*b_{k-1}
      grads = W0^T a0
  Hessian diagonal (exact identity):
      u''_i = -2 * sum_k (h_k . a_k) . (z'_k,i)^2
  Layer-0 term uses constant -2*(W0[i,:])^2 folded into a matmul stationary.

Device layout: activations [feat(part 128 x 2 chunks), batch(free N=256)],
fp16 in SBUF (weights fp16 stationaries; layer-0 stays f32r x f32r since
the PE forbids mixing 32-bit with 16-bit operands); PSUM f32 for matmul
outputs. One [7,N] PSUM accumulator per tile holds rows (hess0..2, u,
grad0..2); all accumulator matmuls write the full [7,N] region at base
partition 0 through zero-padded selector stationaries (col j of 7 selects
the output row), so a single start=True on the first dot-product matmul
replaces bank-clearing.

(1-h^2) factors are fused into consumers as (s-1)*x via
scalar_tensor_tensor (s = h^2): the resulting sign flips alternate per
layer, are absorbed by the q = zd^2 squares on the tangent path, and on
the backward path are folded into per-layer +/-2 dot-product selector
constants — t_k tensors (k>=1) are never materialized.

Engine assignment: Act = tanh x4 + q-squares x9 + output staging; DVE =
tangent/backward PSUM drains (hd, a), products (m, s), a3; Pool(GpSimd,
SBUF-only) = cc products + c0; PE = 86 matmuls/tile at 256-moving each.
Emission is software-pipelined: the tangent ladder is split per direction
(zd pool bufs=4, one bank each) so the three ladders hide each other's
DVE latency, and phase_b (backward + dot products) of tile t-2 is
interleaved chunk-wise into phase_a of tile t via generators.

Boundary overlap (-4% vs the plain loop): all of x is prefetched in one
DMA issued before the weight tables (the startup DMA queue otherwise
stalls tile 0 by ~5us); 4 dummy matmuls during that DMA wait start the
PE P-state ramp clock early; phase_a(0)/phase_a(1) interleave in a fill
prologue with B1(0) driven between their last chunks; the two drain-tail
phase_b generators alternate chunk-by-chunk instead of running serially.
"""

import numpy as np
from contextlib import ExitStack

import concourse.bass as bass
import concourse.bacc as bacc
import concourse.tile as tile
import concourse.mybir as mybir
from concourse.bass_utils import run_bass_kernel_spmd

F32 = mybir.dt.float32
F32R = mybir.dt.float32r
FP16 = mybir.dt.float16
AF = mybir.ActivationFunctionType
ALU = mybir.AluOpType

B, D, H = 65536, 3, 256
NCORES = 8
BLOC = B // NCORES          # 8192 samples per core
N = 256                     # batch tile width (free dim)
NT = BLOC // N              # tiles per core

_CACHE = {}


def _build(nt=NT, reps=1):
    NT_local = nt
    nc = bacc.Bacc("TRN2")

    bloc = NT_local * N
    xt = nc.dram_tensor("xt", [D, bloc], F32R, kind="ExternalInput")
    w0 = nc.dram_tensor("w0", [D, H], F32R, kind="ExternalInput")
    wf = nc.dram_tensor("wf", [128, 3, 2, 2, 128], FP16, kind="ExternalInput")
    wt1 = nc.dram_tensor("wt1", [128, 3, 2, 2, 128], FP16, kind="ExternalInput")
    wb = nc.dram_tensor("wb", [128, 3, 2, 2, 128], FP16, kind="ExternalInput")
    # [7,N]-accumulator stationaries: col i of 7 selects the output row.
    # m2sel[:, i, :]: col i = -2 (hess dir i); wosel[:, c, :]: col 3 = Wout
    # chunk c (u); q0sel[:, c, :]: cols 0..2 = -2*(W0^2)^T (layer-0 hess);
    # w0sel[:, c, :]: cols 4..6 = W0^T (grads).
    m2sel = nc.dram_tensor("m2sel", [128, 2, 3, 7], FP16, kind="ExternalInput")
    wosel = nc.dram_tensor("wosel", [128, 2, 7], FP16, kind="ExternalInput")
    q0sel = nc.dram_tensor("q0sel", [128, 2, 7], FP16, kind="ExternalInput")
    w0sel = nc.dram_tensor("w0sel", [128, 2, 7], FP16, kind="ExternalInput")
    won = nc.dram_tensor("won", [128, 2], F32, kind="ExternalInput")      # -Wout (f32)
    wop = nc.dram_tensor("wop", [128, 2], F32, kind="ExternalInput")      # +Wout (f32)
    out = nc.dram_tensor("out", [7, bloc], F32, kind="ExternalOutput")

    with tile.TileContext(nc) as tc, ExitStack() as ctx:
        const = ctx.enter_context(tc.tile_pool(name="const", bufs=1))
        act = ctx.enter_context(tc.tile_pool(name="act", bufs=2))
        pz = ctx.enter_context(tc.tile_pool(name="pz", bufs=2, space="PSUM"))
        ps_zd = ctx.enter_context(tc.tile_pool(name="ps_zd", bufs=4, space="PSUM"))
        ps_acc = ctx.enter_context(tc.tile_pool(name="ps_acc", bufs=2, space="PSUM"))

        xall = const.tile([D, NT_local * N], F32R)
        nc.sync.dma_start(xall[:], xt[:])
        w0s = const.tile([D, H], F32R)
        nc.sync.dma_start(w0s[:], w0[:])
        wfs = const.tile([128, 3, 2, 2, 128], FP16)
        nc.sync.dma_start(wfs[:], wf[:])
        wt1s = const.tile([128, 3, 2, 2, 128], FP16)
        nc.sync.dma_start(wt1s[:], wt1[:])
        wbs = const.tile([128, 3, 2, 2, 128], FP16)
        nc.sync.dma_start(wbs[:], wb[:])
        m2sels = const.tile([128, 2, 3, 7], FP16)
        nc.sync.dma_start(m2sels[:], m2sel[:])
        wosels = const.tile([128, 2, 7], FP16)
        nc.sync.dma_start(wosels[:], wosel[:])
        q0sels = const.tile([128, 2, 7], FP16)
        nc.sync.dma_start(q0sels[:], q0sel[:])
        w0sels = const.tile([128, 2, 7], FP16)
        nc.sync.dma_start(w0sels[:], w0sel[:])
        wons = const.tile([128, 2], F32)
        nc.sync.dma_start(wons[:], won[:])
        wops = const.tile([128, 2], F32)
        nc.sync.dma_start(wops[:], wop[:])

        def tt(eng, out_ap, in0, in1, op=ALU.mult):
            eng.tensor_tensor(out=out_ap, in0=in0, in1=in1, op=op)

        def sm1(eng, out_ap, s_in, in1):
            # out = (s - 1) * in1   (= -(1-s)*in1; sign tracked by caller)
            eng.scalar_tensor_tensor(out=out_ap, in0=s_in, scalar=1.0, in1=in1,
                                     op0=ALU.subtract, op1=ALU.mult)

        # PE warm-up: dummy matmuls during the initial DMA wait start the
        # P-state ramp clock early (PE would otherwise idle until x lands)
        warm = const.tile([128, 256], FP16)
        nc.vector.memset(warm[:], 0.0)
        warm_ps = pz.tile([128, 2, 256], F32, name="warm_ps", tag="pz")
        for _ in range(4):
            nc.tensor.matmul(warm_ps[:, 0, :], warm[:, 0:128], warm[:],
                             start=True, stop=True)

        state = {}

        def phase_a(it):
            """forward + tangents of tile it. Generator: yields after each
            tangent-layer chunk so phase_b(it-1) work interleaves into the
            per-engine instruction streams; final state lands in state[it]."""
            col = slice(it * N, (it + 1) * N)
            xtile = xall[:, col]

            z = pz.tile([128, 2, N], F32, name="z0", tag="pz")
            for mc in range(2):
                nc.tensor.matmul(z[:, mc, :], w0s[:, mc * 128:(mc + 1) * 128],
                                 xtile, start=True, stop=True)
            h = [None] * 4
            t = [None] * 3
            q = [None] * 4
            h[0] = act.tile([128, 2, N], FP16, name="h0", tag="h0", bufs=6)
            nc.scalar.activation(h[0][:], z[:], AF.Tanh)
            s = [None] * 4
            s[0] = act.tile([128, 2, N], FP16, name="s0", tag="s0", bufs=6)
            tt(nc.vector, s[0][:], h[0][:], h[0][:])
            t[0] = act.tile([128, 2, N], FP16, name="t0", tag="t0", bufs=6)
            nc.vector.tensor_scalar(out=t[0][:], in0=s[0][:], scalar1=-1.0,
                                    scalar2=1.0, op0=ALU.mult, op1=ALU.add)

            hd = None
            a3 = None
            for k in (1, 2, 3):
                l = k - 1
                z = pz.tile([128, 2, N], F32, name=f"z{k}", tag="pz")
                for mc in range(2):
                    for kc in range(2):
                        nc.tensor.matmul(z[:, mc, :], wfs[:, l, kc, mc, :],
                                         h[k - 1][:, kc, :],
                                         start=(kc == 0), stop=(kc == 1))
                h[k] = act.tile([128, 2, N], FP16, name=f"h{k}", tag=f"h{k}", bufs=6)
                nc.scalar.activation(h[k][:], z[:], AF.Tanh)
                if k < 3:
                    s[k] = act.tile([128, 2, N], FP16, name=f"s{k}", tag=f"s{k}", bufs=6)
                    if k == 1:
                        nc.scalar.activation(s[k][:], h[k][:], AF.Square)
                    else:
                        tt(nc.vector, s[k][:], h[k][:], h[k][:])
                else:
                    s[3] = act.tile([128, 2, N], FP16, name="s3", tag="s3", bufs=6)
                    tt(nc.gpsimd, s[3][:], h[3][:], h[3][:])

                zds = []
                new_hd = [None] * 3
                q[k] = act.tile([128, 3, 2, N], FP16, name=f"q{k}", tag=f"q{k}", bufs=4)
                for i in range(3):
                    zd = ps_zd.tile([128, 2, N], F32, name=f"zd{k}_{i}", tag="zd")
                    zds.append(zd)
                    for mc in range(2):
                        for kc in range(2):
                            if k == 1:
                                nc.tensor.matmul(zd[:, mc, :],
                                                 wt1s[:, i, kc, mc, :],
                                                 t[0][:, kc, :],
                                                 start=(kc == 0), stop=(kc == 1))
                            else:
                                nc.tensor.matmul(zd[:, mc, :],
                                                 wfs[:, l, kc, mc, :],
                                                 hd[i][:, kc, :],
                                                 start=(kc == 0), stop=(kc == 1))
                for i in range(3):
                    nc.scalar.activation(q[k][:, i, :, :], zds[i][:], AF.Square)
                    if k < 3:
                        # hd' = (s_k - 1) * zd  (sign alternates; q squares absorb it)
                        new_hd[i] = act.tile([128, 2, N], FP16,
                                             name=f"hd{k}_{i}", tag=f"hd{i}", bufs=4)
                        sm1(nc.vector, new_hd[i][:], s[k][:], zds[i][:])
                hd = new_hd
                if k == 3:
                    state[it] = (col, h, s, q)
                yield

        def phase_b(it, last=False):
            """backward + dot products of tile it. Generator: 4 chunks.
            Sign ledger: a3'=-a3, a2'=+a2, a1'=-a1, a0'=+a0 (from the (s-1)
            fusion); cc_k inherits a_k's sign; m2sel[:,0] = +2 (k=3,1),
            m2sel[:,1] = -2 (k=2)."""
            col, h, s, q = state.pop(it)
            # a3' = -t3*Wout = s3*w - w  (negated; absorbed in m2sel signs)
            a3 = act.tile([128, 2, N], FP16, name="a3", tag="a3", bufs=6)
            for c in range(2):
                nc.vector.tensor_scalar(out=a3[:, c, :], in0=s[3][:, c, :],
                                        scalar1=wops[:, c:c + 1],
                                        scalar2=wons[:, c:c + 1],
                                        op0=ALU.mult, op1=ALU.add)
            a = a3
            if last:
                # drain tail: borrow an idle zd bank so the two final tiles'
                # dot-product groups don't serialize on the single acc bank
                acc = ps_zd.tile([7, N], F32, name="acc_last", tag="zd")
            else:
                acc = ps_acc.tile([7, N], F32, name="acc", tag="acc")
            first = True
            for k in (3, 2, 1):
                cc = act.tile([128, 2, N], FP16, name=f"cc{k}", tag="cc", bufs=6)
                tt(nc.gpsimd, cc[:], h[k][:], a[:])
                ccb = cc[:].unsqueeze(1).broadcast_to([128, 3, 2, N])
                m = act.tile([128, 3, 2, N], FP16, name=f"m{k}", tag=f"m{k}", bufs=4)
                tt(nc.vector, m[:], q[k][:], ccb)
                bk = pz.tile([128, 2, N], F32, name=f"bk{k}", tag="pz")
                for mc in range(2):
                    for kc in range(2):
                        nc.tensor.matmul(bk[:, mc, :], wbs[:, k - 1, kc, mc, :],
                                         a[:, kc, :],
                                         start=(kc == 0), stop=(kc == 1))
                a = act.tile([128, 2, N], FP16, name=f"a{k - 1}", tag="a", bufs=6)
                sm1(nc.vector, a[:], s[k - 1][:], bk[:])
                # drain this k's dot products into acc right away
                sgn = 0 if k != 2 else 1
                for i in range(3):
                    for c in range(2):
                        nc.tensor.matmul(acc[:], m2sels[:, sgn, i, :],
                                         m[:, i, c, :],
                                         start=first, stop=False,
                                         skip_group_check=True)
                        first = False
                if k == 3:
                    for c in range(2):
                        nc.tensor.matmul(acc[:], wosels[:, c, :], h[3][:, c, :],
                                         start=False, stop=False,
                                         skip_group_check=True)
                yield
            c0 = act.tile([128, 2, N], FP16, name="c0", tag="c0", bufs=4)
            tt(nc.gpsimd, c0[:], h[0][:], a[:])

            for c in range(2):
                nc.tensor.matmul(acc[:], q0sels[:, c, :], c0[:, c, :],
                                 start=False, stop=False,
                                 skip_group_check=True)
            for c in range(2):
                nc.tensor.matmul(acc[:], w0sels[:, c, :], a[:, c, :],
                                 start=False, stop=(c == 1),
                                 skip_group_check=True)

            stg = act.tile([7, N], F32, name="stg", tag="stg", bufs=4)
            nc.scalar.copy(stg[:], acc[:])
            nc.sync.dma_start(out[:, col], stg[:])

        def drive(gen):
            if gen is None:
                return None
            try:
                next(gen)
                return gen
            except StopIteration:
                return None

        for rep in range(reps):
            queue = {}
            # fill prologue: interleave A(0) and A(1) chunk-by-chunk so each
            # fills the other's dependency-chain gaps (no B partner exists yet)
            if NT_local >= 2:
                ga0, ga1 = phase_a(0), phase_a(1)
                drive(ga0)
                drive(ga1)
                drive(ga0)
                drive(ga1)
                drive(ga0)
                gb0 = phase_b(0)
                drive(gb0)          # B1(0) between A3(0) and A3(1)
                drive(ga1)
                queue[0] = gb0
                queue[1] = phase_b(1)
                start_it = 2
            else:
                start_it = 0
            for it in range(start_it, NT_local):
                ga = phase_a(it)
                gb = queue.pop(it - 2, None)
                for _ in range(3):          # 3 layer chunks of A
                    drive(ga)
                    gb = drive(gb)
                gb = drive(gb)              # B final chunk (dd+stg)
                assert gb is None
                queue[it] = phase_b(it, last=(it >= NT_local - 2))
            gens = [queue[it] for it in sorted(queue)]
            while gens:
                gens = [g for g in (drive(g) for g in gens) if g is not None]

    nc.compile()
    return nc


def _host_pack(inputs):
    x = np.ascontiguousarray(np.asarray(inputs["x"], np.float32))
    W = [np.asarray(inputs[f"W{i}"], np.float32) for i in range(4)]
    Wout = np.asarray(inputs["Wout"], np.float32)
    bout = np.asarray(inputs["bout"], np.float32)

    def pack_w(w):   # [256,256] -> [128, 2(kc), 2(mc), 128]
        return np.ascontiguousarray(w.reshape(2, 128, 2, 128).transpose(1, 0, 2, 3))

    wf = np.ascontiguousarray(np.stack([pack_w(W[1]), pack_w(W[2]), pack_w(W[3])], axis=1))
    wbk = np.ascontiguousarray(np.stack(
        [pack_w(W[1].T.copy()), pack_w(W[2].T.copy()), pack_w(W[3].T.copy())], axis=1))

    def pack_kd(wkd):   # [256, D] -> [128, 2, D]
        return np.ascontiguousarray(wkd.reshape(2, 128, D).transpose(1, 0, 2))

    w0t = pack_kd(W[0].T.copy())                      # [128, 2, 3] grads lhsT
    q0t = pack_kd((-2.0 * W[0].astype(np.float64) ** 2).astype(np.float32).T.copy())
    wt1 = np.ascontiguousarray(np.stack(
        [pack_w((W[0][i, :][:, None] * W[1]).astype(np.float32)) for i in range(3)],
        axis=1))
    wo = np.ascontiguousarray(Wout[:, 0].reshape(2, 128).T)     # [128, 2]
    xtp = np.ascontiguousarray(x.T)                             # [D, B]

    # [7,N]-accumulator selector stationaries (see _build for row layout).
    # m2sel[:, 0]: +2 (layers 3,1 whose cc carries a negated a');
    # m2sel[:, 1]: -2 (layer 2, true-sign cc).
    m2sel = np.zeros((128, 2, 3, 7), np.float16)
    for i in range(3):
        m2sel[:, 0, i, i] = 2.0
        m2sel[:, 1, i, i] = -2.0
    wosel = np.zeros((128, 2, 7), np.float32)
    wosel[:, :, 3] = wo
    q0sel = np.zeros((128, 2, 7), np.float32)
    q0sel[:, :, 0:3] = q0t
    w0sel = np.zeros((128, 2, 7), np.float32)
    w0sel[:, :, 4:7] = w0t

    shared = dict(w0=W[0], wf=wf.astype(np.float16), wb=wbk.astype(np.float16),
                  wt1=wt1.astype(np.float16), m2sel=m2sel,
                  wosel=wosel.astype(np.float16), q0sel=q0sel.astype(np.float16),
                  w0sel=w0sel.astype(np.float16),
                  won=np.ascontiguousarray(-wo), wop=wo.copy())
    return xtp, shared, float(bout[0])


LAST_EXEC_NS = None


def kernel(**inputs):
    global LAST_EXEC_NS
    import os
    if "nc" not in _CACHE:
        _CACHE["nc"] = _build()
    nc = _CACHE["nc"]

    xt, shared, bout = _host_pack(inputs)
    in_maps = []
    for c in range(NCORES):
        m = dict(shared)
        m["xt"] = np.ascontiguousarray(xt[:, c * BLOC:(c + 1) * BLOC])
        in_maps.append(m)

    trace = bool(int(os.environ.get("BASS_PINN_TRACE", "0")))
    res = run_bass_kernel_spmd(nc, in_maps, core_ids=list(range(NCORES)),
                               trace=trace)
    if res.exec_time_ns is not None:
        LAST_EXEC_NS = res.exec_time_ns
    if res.instructions_and_trace is not None:
        print("trace:", res.instructions_and_trace[1])
    full = np.concatenate([res.results[c]["out"] for c in range(NCORES)], axis=1)
    y = np.empty((full.shape[1], 7), np.float32)
    y[:, 0] = full[3] + np.float32(bout)
    y[:, 1:4] = full[4:7].T
    y[:, 4:7] = full[0:3].T
    return y

